# revision 36
# baseline (speedup 1.0000x reference)
"""DPOT2D layer (AFNO-style) Trainium2 kernel.

out = x + irfft2_pad(blockMLP(trunc64(rfft2(x))))   (ortho norm)

Sharding: tensor-parallel over the 8 block-diagonal channel groups — core n
gets channels [n*64, (n+1)*64) and its block's MLP weights. Blocks never mix,
so there is zero cross-core communication.

Per core, every FFT stage is a DFT matmul on the TensorEngine (bf16 operands,
fp32 PSUM accumulation), with PE-transpose corner turns between stages:

  A:  U[k1s,(w,c)]      = F_h^T @ x        (contract h, 2x128 K-chunks)
  t1: V[w,(k1s,c)]      = corner turn of U (per-channel 128x128 transposes)
  B:  Y[k2s,(k1,c)]     = DFT_w on complex U (re/im column accumulation)
  t2: Yt[(k1b,c),...]   = PAIRED turn: each transpose takes a k1 pair, so
                          all 128 out partitions are used (k1'=k1b*32+pair)
  L1: o1 = gelu(W1 Y + b1)                 (K=64 at base 0/64 per k1-bit)
  L2: O2[(o2r|o2i),(k1',k2)] = W2 o1 + b2  (K=128)
  t3: R[(k1''b,k2),...] = paired turn of O2 (k1'' = second re-pairing)
  iW: G[w,(j,k1'',c')]  = hermitian irfft_w matmuls (re/im accumulation)
  t4: Ght[(j,k1''),(w,c')] = corner turn of G (re/im -> k1-stack)
  iH: x'[h,(w,c')]      = lih_perm^T @ Ght (+ residual), DMA out

The double k1 re-pairing of t2/t3 is undone on the host by permuting the
rows of lih (k1 = g(g(k1'')), g(v) = 2*(v%32) + v//32). The residual add is
split: half the output tiles via DVE tensor_tensor, half via PE
identity-matmul accumulate + ACT copy, to balance the three engines.
Evacuations use 2-bank (1024-col) PSUM tiles; PSUM -> SBUF copies rotate
DVE/ACT; input loads split across both HWDGE rings; stores go on the SWDGE
(gpsimd) queue; all matrix constants arrive in a single packed DMA.

I/O precision: x is loaded once as bf16 (8 MB/core) and kept resident for the
residual add; the output is stored as bf16 (8 MB/core) and upcast to f32 on
the host. The spectral correction is ~3e-5 of the output norm, so total
rel-err is dominated by bf16 rounding of x (~2e-3), well inside the 2e-2 gate.
"""

import numpy as np
import ml_dtypes

import concourse.bass as bass
import concourse.mybir as mybir
from concourse import bacc
from concourse import masks
from concourse.tile import TileContext
from concourse.bass_utils import run_bass_kernel_spmd

B = 2
H = 256
W = 256
C = 512
NB = 8
BS = 64          # channels per block (= per core)
KEEP = 64        # kept modes per spatial dim
HID = 128
P = 128

BF16 = mybir.dt.bfloat16
FP8 = mybir.dt.float8e4
F32 = mybir.dt.float32
AF = mybir.ActivationFunctionType

_CACHED_NC = None


def _host_consts():
    """DFT matrices shared by all cores (fp32 -> bf16)."""
    h = np.arange(H, dtype=np.float64)[:, None]
    k = np.arange(KEEP, dtype=np.float64)[None, :]
    th = 2.0 * np.pi * h * k / H
    F = np.concatenate([np.cos(th), -np.sin(th)], axis=1) / 16.0      # (256,128)
    Fwre, Fwim = F[:, :KEEP], F[:, KEEP:]
    lb_re = np.concatenate([Fwre, Fwim], axis=1)                      # (256,128)
    lb_im = np.concatenate([-Fwim, Fwre], axis=1)
    alpha = np.where(np.arange(KEEP) == 0, 1.0, 2.0)
    k2 = np.arange(KEEP, dtype=np.float64)[:, None]
    wv = np.arange(W, dtype=np.float64)[None, :]
    tw = 2.0 * np.pi * k2 * wv / W
    Ca = alpha[:, None] * np.cos(tw) / 16.0                           # (64,256)
    Sa = alpha[:, None] * np.sin(tw) / 16.0
    k1 = np.arange(KEEP, dtype=np.float64)[:, None]
    hv = np.arange(H, dtype=np.float64)[None, :]
    tih = 2.0 * np.pi * k1 * hv / H
    Ehc = np.cos(tih) / 16.0                                          # (64,256)
    Ehs = np.sin(tih) / 16.0
    lih_full = np.concatenate([Ehc, -Ehs], axis=0)                    # (128,256)

    bf = ml_dtypes.bfloat16
    ffwd = np.stack([F[0:128], F[128:256]]).astype(bf)                # (2,128,128)
    lbw = np.stack([
        np.stack([lb_re[0:128], lb_im[0:128]]),
        np.stack([lb_re[128:256], lb_im[128:256]]),
    ]).astype(bf)                                                     # (2,2,128,128)
    liw = np.stack([
        np.stack([np.stack([Ca[:, 0:128], -Sa[:, 0:128]]),
                  np.stack([Sa[:, 0:128], Ca[:, 0:128]])]),
        np.stack([np.stack([Ca[:, 128:256], -Sa[:, 128:256]]),
                  np.stack([Sa[:, 128:256], Ca[:, 128:256]])]),
    ]).astype(bf)                                                     # (2,2,2,64,128)
    lih = np.stack([lih_full[:, 0:128], lih_full[:, 128:256]]).astype(bf)  # (2,128,128)
    return ffwd, lbw, liw, lih


def _build_nc(loop_iters=0, probe=None):
    """loop_iters>0 wraps the whole per-batch pipeline in an on-device
    For_i repeat loop — used only by the timing harness to amortize the
    ~80ms axon dispatch overhead out of the measurement.
    probe: None | 'dma' (DMAs only) | 'compute' (token input/output DMAs)."""
    nc = bacc.Bacc()

    xbf = nc.declare_dram_parameter("xbf", [B, H, W, BS], BF16, isOutput=False)
    # all bf16 matrix constants packed into one tensor -> one DMA:
    # cols 0:256 FW, 256:768 LBW, 768:1024 M2, 1024:1280 LIH (128 rows);
    # cols 1280:1792 M1, 1792:2816 LIW (rows 0:64).
    cb_d = nc.declare_dram_parameter("cb", [P, 2816], BF16, isOutput=False)
    bias_d = nc.declare_dram_parameter("bias", [P, 3], F32, isOutput=False)
    cbf8_d = nc.declare_dram_parameter("cbf8", [P, 768], FP8, isOutput=False)
    out = nc.declare_dram_parameter("out", [B, H, W, BS], BF16, isOutput=True)

    with TileContext(nc) as tc:
        consts = tc.alloc_tile_pool(name="consts", bufs=1)
        ident = consts.tile([P, P], BF16, name="ident")
        masks.make_identity(nc, ident[:])

        # consts go on the scalar ring so they overlap the first x loads
        # (which go on the sync ring).
        cb = consts.tile([P, 2816], BF16, name="cb")
        nc.scalar.dma_start(out=cb[:], in_=cb_d[:])
        bias_t = consts.tile([P, 3], F32, name="bias")
        nc.scalar.dma_start(out=bias_t[:], in_=bias_d[:])
        cbf8 = consts.tile([P, 768], FP8, name="cbf8")
        nc.scalar.dma_start(out=cbf8[:], in_=cbf8_d[:])

        FW = [cb[:, hh * P:(hh + 1) * P] for hh in range(2)]
        LBW = [[cb[:, 256 + (wh * 2 + s) * P:256 + (wh * 2 + s + 1) * P]
                for s in range(2)] for wh in range(2)]
        LBW8 = [cbf8[:, s * 256:(s + 1) * 256].rearrange("p (t m) -> p t m", t=2)
                for s in range(2)]
        FW8 = cbf8[:, 512:768].rearrange("p (t m) -> p t m", t=2)
        M2 = [cb[:, 768 + s * P:768 + (s + 1) * P] for s in range(2)]
        LIH = [cb[:, 1024 + hc * P:1024 + (hc + 1) * P] for hc in range(2)]
        # M1/LIW are stored twice (rows 0:64 and 64:128) so the stationary
        # operand can sit at either partition base for the paired corner
        # turns; index [k1bit] selects the base.
        M1 = [[[cb[bit * BS:(bit + 1) * BS,
                   1280 + (j * 2 + s) * P:1280 + (j * 2 + s + 1) * P]
                for bit in range(2)] for s in range(2)] for j in range(2)]
        LIW = [[[[cb[bit * KEEP:(bit + 1) * KEEP,
                     1792 + ((wh * 2 + j) * 2 + s) * P:
                     1792 + ((wh * 2 + j) * 2 + s + 1) * P]
                  for bit in range(2)] for s in range(2)]
                 for j in range(2)] for wh in range(2)]
        b1s_t = [bias_t[0:HID, j:j + 1] for j in range(2)]
        b2s_t = bias_t[:, 2:3]

        # copy-engine rotation (PSUM-capable engines only: DVE + ACT).
        # DVE also carries the invH residual adds, ACT the gelu/bias work.
        cp_cnt = [0]
        CP_PATTERN = globals().get("CP_PATTERN_OVERRIDE") or ("v", "a")

        def cp(dst, src):
            kind = CP_PATTERN[cp_cnt[0] % len(CP_PATTERN)]
            cp_cnt[0] += 1
            if kind == "v":
                nc.vector.tensor_copy(out=dst, in_=src)
            else:
                nc.scalar.activation(out=dst, in_=src, func=AF.Copy)

        # Tag-sharing across stage lifetimes keeps SBUF within budget:
        #   tagA/tagB: U[wh] -> G[wh]   tagC/tagD: V[wh] -> Ght[wh]
        #   tagE: Y -> R                tagF: Yt -> O2
        sb = tc.alloc_tile_pool(name="sb", bufs=1)
        xin = tc.alloc_tile_pool(name="xin", bufs=1)
        outp = tc.alloc_tile_pool(name="outp", bufs=2)
        pmm = tc.alloc_tile_pool(name="pmm", bufs=3, space="PSUM")
        ptp = tc.alloc_tile_pool(name="ptp", bufs=2, space="PSUM")

        import contextlib
        loop_ctx = tc.For_i(0, loop_iters, 1) if loop_iters else contextlib.nullcontext()
        with loop_ctx:
            _emit_body(nc, tc, locals(), probe=probe)
        ptp.release()
        pmm.release()
        outp.release()
        xin.release()
        sb.release()
        consts.release()
    nc.compile()
    return nc


_RES_MODE = (True, False)


def _emit_body(nc, tc, env, probe=None):
    xbf = env["xbf"]; out = env["out"]
    FW = env["FW"]; LBW = env["LBW"]; LBW8 = env["LBW8"]; FW8 = env["FW8"]
    M1 = env["M1"]; M2 = env["M2"]
    LIW = env["LIW"]; LIH = env["LIH"]; b1s_t = env["b1s_t"]; b2s_t = env["b2s_t"]
    ident = env["ident"]; cp = env["cp"]; cp_cnt = env["cp_cnt"]
    sb = env["sb"]; xin = env["xin"]; outp = env["outp"]
    pmm = env["pmm"]; ptp = env["ptp"]

    dma_only = probe == "dma"
    no_io = probe == "compute"

    for b in range(B):
        # -------- input loads: 8 x 1MB [128h, 64w, 64c], resident all batch.
        # wq-major order and hh split across the two HWDGE rings so stage A
        # can start after the first (wq=0) pair lands.
        xt = [[None] * 4 for _ in range(2)]
        for wq in range(4):
            for hh in range(2):
                t = xin.tile([P, 64, BS], BF16, tag=f"xin{hh}{wq}",
                             name=f"xin{hh}{wq}_{b}")
                eng = nc.sync if hh == 0 else nc.scalar
                if no_io:
                    eng.dma_start(out=t[0:1, 0:1, :], in_=xbf[b, 0:1, 0:1, :])
                else:
                    eng.dma_start(
                        out=t[:],
                        in_=xbf[b, hh * P:(hh + 1) * P, wq * 64:(wq + 1) * 64, :])
                xt[hh][wq] = t
        xf = [None] * 4
        for wq in range(4):
            t8 = xin.tile([P, 2, 64, BS], FP8, tag=f"xf8{wq}",
                          name=f"xf8{wq}_{b}")
            for hh in range(2):
                if no_io:
                    nc.gpsimd.dma_start(out=t8[0:1, hh, 0:1, :],
                                        in_=xbf[b, 0:1, 0:1, :])
                else:
                    nc.gpsimd.dma_start(
                        out=t8[:, hh, :, :],
                        in_=xbf[b, hh * P:(hh + 1) * P, wq * 64:(wq + 1) * 64, :])
            xf[wq] = t8
        if dma_only:
            # stores fed straight from the input tiles: same DMA traffic,
            # no compute.
            for hc in range(2):
                for wq in range(4):
                    nc.gpsimd.dma_start(
                        out=out[b, hc * P:(hc + 1) * P, wq * 64:(wq + 1) * 64, :],
                        in_=xt[hc][wq][:])
            continue

        # ---------------- stage A: U[wh] (128=k1s, (w 128, c 64)) ----------
        # 2-bank PSUM tiles: two N=512 accumulation groups -> one 1024-col
        # evacuation.
        U = [sb.tile([P, 128, BS], BF16, tag=f"tagAB{wh}", name=f"U{wh}_{b}")
             for wh in range(2)]
        for wq in range(4):          # w chunks of 64
            for mm in range(4):      # 16 w each -> 1024 cols
                ps = pmm.tile([P, 2, 8, BS], F32, tag="mm",
                              name=f"psA_{b}_{wq}_{mm}")
                for half in range(2):
                    w0 = (mm * 2 + half) * 8
                    nc.tensor.matmul(
                        ps[:, half, :, :], FW8,
                        xf[wq][:, :, w0:w0 + 8, :],
                        start=True, stop=True,
                        perf_mode=mybir.MatmulPerfMode.DoubleRow)
                wg = wq * 2 + mm // 2            # 16-w group (0..7)
                cp(U[wg // 4][:, (wg % 4) * 32 + (mm % 2) * 16:
                              (wg % 4) * 32 + (mm % 2) * 16 + 16, :], ps[:])

        # ---------------- turn1: V (128=w, (wh 2, k1s 128, c 64), fp8) -----
        # 8 transposes share one 1-bank bf16 PSUM tile -> 1024-col evac.
        # V is one fp8 tile; the wh axis is the DoubleRow k-tile dim of
        # stage B, and the copy casts bf16 -> fp8 for free.
        V = sb.tile([P, 2, P, BS], FP8, tag="tagCD0", name=f"V_{b}")
        for cg in range(8):          # groups of 8 channels; wh inner so that
            for wh in range(2):      # stage B's c-chunks unblock early
                pt = ptp.tile([P, 8, P], BF16, tag="tp", name=f"t1_{b}_{wh}_{cg}")
                for i in range(8):
                    nc.tensor.transpose(pt[:, i, :], U[wh][:, :, cg * 8 + i],
                                        ident[:])
                cp(V[:, wh, :, cg * 8:cg * 8 + 8],
                   pt[:, :, :].rearrange("a c k -> a k c"))

        # ---------------- stage B: Y (128=k2s, (k1 64, c 64)) --------------
        Y = sb.tile([P, KEEP, BS], BF16, tag="tagE", name=f"Y_{b}")
        for mm in range(4):          # 16 k1 per tile -> 1024 cols
            ps = pmm.tile([P, 2, 8, BS], F32, tag="mm", name=f"psB_{b}_{mm}")
            for s in range(2):       # 0: re rows (k1s 0:64), 1: im rows
                for half in range(2):
                    k0 = (mm * 2 + half) * 8
                    rhs = V[:, :, s * KEEP + k0:s * KEEP + k0 + 8, :]
                    nc.tensor.matmul(ps[:, half, :, :], LBW8[s], rhs,
                                     start=(s == 0), stop=(s == 1),
                                     perf_mode=mybir.MatmulPerfMode.DoubleRow)
            cp(Y[:, mm * 16:(mm + 1) * 16, :], ps[:])

        # ---------------- turn2: Yt ((k1bit,c)=128, (k1pair 32, k2s 128)) --
        # Full-width turns: each transpose takes a k1 PAIR (free dims
        # (k1 2, c 64) -> 128 out partitions). Storage k1' = k1bit*32+pair,
        # true k1 = 2*pair + k1bit; undone by the host-side lih permutation.
        Yt = sb.tile([P, 32, P], BF16, tag="tagF", name=f"Yt_{b}")
        Yf = Y[:, :, :].rearrange("a k c -> a (k c)")
        for kg in range(4):          # groups of 8 pairs
            pt = ptp.tile([P, 8, P], BF16, tag="tp", name=f"t2_{b}_{kg}")
            for i in range(8):
                q = kg * 8 + i
                nc.tensor.transpose(pt[:, i, :], Yf[:, q * P:(q + 1) * P],
                                    ident[:])
            cp(Yt[:, kg * 8:kg * 8 + 8, :], pt[:])

        # ---------------- MLP L1 (K=64) + gelu -----------------------------
        # o1 storage index k1' = k1bit*32 + pair.
        o1 = [sb.tile([HID, 2, 32, KEEP], BF16, tag=f"o1_{j}", name=f"o1_{j}_{b}")
              for j in range(2)]
        for j in range(2):
            for bit in range(2):
                for mm in range(2):  # 16 pairs per tile -> 1024 cols
                    ps = pmm.tile([HID, 2, 8, KEEP], F32, tag="mm",
                                  name=f"ps1_{b}_{j}_{bit}_{mm}")
                    for s in range(2):
                        for half in range(2):
                            p0 = (mm * 2 + half) * 8
                            nc.tensor.matmul(
                                ps[:, half, :, :], M1[j][s][bit],
                                Yt[bit * BS:(bit + 1) * BS, p0:p0 + 8,
                                   s * KEEP:(s + 1) * KEEP],
                                start=(s == 0), stop=(s == 1))
                    nc.scalar.activation(
                        out=o1[j][:, bit, mm * 16:(mm + 1) * 16, :],
                        in_=ps[:], func=AF.Gelu, bias=b1s_t[j])

        # ---------------- MLP L2 (K=128) + bias ----------------------------
        O2 = sb.tile([P, 2, 32, KEEP], BF16, tag="tagF", name=f"O2_{b}")
        for bit in range(2):
            for mm in range(2):
                ps = pmm.tile([P, 2, 8, KEEP], F32, tag="mm",
                              name=f"ps2_{b}_{bit}_{mm}")
                for j in range(2):
                    for half in range(2):
                        p0 = (mm * 2 + half) * 8
                        nc.tensor.matmul(ps[:, half, :, :], M2[j],
                                         o1[j][:, bit, p0:p0 + 8, :],
                                         start=(j == 0), stop=(j == 1))
                if cp_cnt[0] % 2 == 0:
                    nc.vector.tensor_scalar_add(
                        out=O2[:, bit, mm * 16:(mm + 1) * 16, :], in0=ps[:],
                        scalar1=b2s_t)
                else:
                    nc.scalar.activation(
                        out=O2[:, bit, mm * 16:(mm + 1) * 16, :],
                        in_=ps[:], func=AF.Identity, bias=b2s_t)
                cp_cnt[0] += 1

        # ---------------- turn3: R ((k1''bit,k2)=128, (u 32, o2s 128)) -----
        # Pairs along flat k1': storage k1'' = bit*32 + u, k1' = 2u + bit.
        R = sb.tile([P, 32, P], BF16, tag="tagE", name=f"R_{b}")
        O2f = O2[:, :, :, :].rearrange("a b p k -> a (b p k)")
        for kg in range(4):
            pt = ptp.tile([P, 8, P], BF16, tag="tp", name=f"t3_{b}_{kg}")
            for i in range(8):
                u = kg * 8 + i
                nc.tensor.transpose(pt[:, i, :], O2f[:, u * P:(u + 1) * P],
                                    ident[:])
            cp(R[:, kg * 8:kg * 8 + 8, :], pt[:])

        # ---------------- invW: G[wh] (128=w, (j 2, k1'' 64, c' 64)) -------
        G = [sb.tile([P, 2, KEEP, BS], BF16, tag=f"tagAB{wh}", name=f"G{wh}_{b}")
             for wh in range(2)]
        for wh in range(2):
            for j in range(2):       # 0: Gre, 1: Gim
                for mm in range(4):  # 16 k1'' per tile; bit = mm//2
                    bit = mm // 2
                    ps = pmm.tile([P, 2, 8, BS], F32, tag="mm",
                                  name=f"psW_{b}_{wh}_{j}_{mm}")
                    for s in range(2):
                        for half in range(2):
                            u0 = (mm % 2) * 16 + half * 8
                            nc.tensor.matmul(
                                ps[:, half, :, :], LIW[wh][j][s][bit],
                                R[bit * KEEP:(bit + 1) * KEEP, u0:u0 + 8,
                                  s * KEEP:(s + 1) * KEEP],
                                start=(s == 0), stop=(s == 1))
                    cp(G[wh][:, j, mm * 16:(mm + 1) * 16, :], ps[:])

        # ---------------- turn4: Ght (128=k1s, (w 256, c' 64)) -------------
        Ght = [sb.tile([P, P, BS], BF16, tag=f"tagCD{wh}", name=f"Ght{wh}_{b}")
               for wh in range(2)]
        for wh in range(2):
            for cg in range(8):
                pt = ptp.tile([P, 8, P], BF16, tag="tp", name=f"t4_{b}_{wh}_{cg}")
                for i in range(8):
                    # free slice (j 2, k1 64) -> out partitions [k1re | k1im]
                    nc.tensor.transpose(pt[:, i, :], G[wh][:, :, :, cg * 8 + i],
                                        ident[:])
                cp(Ght[wh][:, :, cg * 8:cg * 8 + 8], pt[:, :, :].rearrange(
                    "a b c -> a c b"))

        # ---------------- invH + residual + store --------------------------
        # Residual is added on the PE (identity-matmul accumulate), so the
        # evacuation is a plain copy that rotates across DVE/ACT. wq-major,
        # hc-inner order frees both residual tiles of a wq early, unblocking
        # the next batch's input loads.
        for wq2 in range(8):         # groups of 32 w -> 512KB stores
            wq = wq2 // 2
            for hc in range(2):
                ot = outp.tile([P, 32, BS], BF16, tag="ot",
                               name=f"ot_{b}_{hc}_{wq2}")
                for mm in range(2):  # 16 w per tile -> 1024 cols
                    gmm = (wq2 % 2) * 2 + mm       # 16-w group within wq
                    ps = pmm.tile([P, 2, 8, BS], F32, tag="mm",
                                  name=f"psH_{b}_{hc}_{wq2}_{mm}")
                    # alternate residual-add strategy: DVE tensor_tensor for
                    # half the tiles; PE identity-accumulate + ACT copy for
                    # the other half (balances PE vs DVE vs ACT).
                    on_dve = _RES_MODE[(gmm + hc) % len(_RES_MODE)]
                    for half in range(2):
                        wg = wq * 8 + gmm * 2 + half  # global 8-w group
                        nc.tensor.matmul(
                            ps[:, half, :, :], LIH[hc],
                            Ght[wg // 16][:, (wg % 16) * 8:(wg % 16) * 8 + 8, :],
                            start=True, stop=on_dve)
                    if on_dve:
                        nc.vector.tensor_tensor(
                            out=ot[:, mm * 16:(mm + 1) * 16, :], in0=ps[:],
                            in1=xt[hc][wq][:, gmm * 16:(gmm + 1) * 16, :],
                            op=mybir.AluOpType.add)
                    else:
                        for half in range(2):
                            nc.tensor.matmul(
                                ps[:, half, :, :], ident[:],
                                xt[hc][wq][:, (gmm * 2 + half) * 8:
                                           (gmm * 2 + half + 1) * 8, :],
                                start=False, stop=True)
                        nc.scalar.activation(
                            out=ot[:, mm * 16:(mm + 1) * 16, :], in_=ps[:],
                            func=AF.Copy)
                if no_io:
                    nc.gpsimd.dma_start(out=out[b, 0:1, 0:1, :],
                                        in_=ot[0:1, 0:1, :])
                else:
                    nc.gpsimd.dma_start(
                        out=out[b, hc * P:(hc + 1) * P,
                                wq2 * 32:(wq2 + 1) * 32, :],
                        in_=ot[:])


def _prepare_in_maps(x, w1, b1, w2, b2):
    bf = ml_dtypes.bfloat16
    ffwd, lbw, liw, lih = _host_consts()
    x = np.asarray(x, dtype=np.float32)

    in_maps = []
    for n in range(NB):
        xs = np.ascontiguousarray(x[..., n * BS:(n + 1) * BS])
        w1n = np.asarray(w1[:, n], dtype=np.float32)   # (2,64,128)
        w2n = np.asarray(w2[:, n], dtype=np.float32)   # (2,128,64)
        b1n = np.asarray(b1[:, n], dtype=np.float32)   # (2,128)
        b2n = np.asarray(b2[:, n], dtype=np.float32)   # (2,64)
        m1 = np.stack([
            np.stack([w1n[0], -w1n[1]]),
            np.stack([w1n[1], w1n[0]]),
        ]).astype(bf)                                   # (2,2,64,128)
        m2 = np.stack([
            np.concatenate([w2n[0], w2n[1]], axis=1),
            np.concatenate([-w2n[1], w2n[0]], axis=1),
        ]).astype(bf)                                   # (2,128,128)

        # lih rows are permuted to undo the storage order of the paired
        # corner turns: storage k1'' -> true k1 via two rounds of
        # g(v) = 2*(v%32) + v//32.
        g = lambda v: 2 * (v % 32) + v // 32
        k1true = np.array([g(g(v)) for v in range(KEEP)])
        rowperm = np.concatenate([k1true, KEEP + k1true])

        cb = np.zeros((P, 2816), dtype=bf)
        cb[:, 0:256] = np.concatenate([ffwd[0], ffwd[1]], axis=1)
        cb[:, 256:768] = np.concatenate(
            [lbw[wh, s] for wh in range(2) for s in range(2)], axis=1)
        cb[:, 768:1024] = np.concatenate([m2[0], m2[1]], axis=1)
        cb[:, 1024:1280] = np.concatenate(
            [lih[0][rowperm], lih[1][rowperm]], axis=1)
        m1cat = np.concatenate(
            [m1[j, s] for j in range(2) for s in range(2)], axis=1)
        cb[0:BS, 1280:1792] = m1cat
        cb[BS:P, 1280:1792] = m1cat          # duplicate for base-64 operand
        liwcat = np.concatenate(
            [liw[wh, j, s] for wh in range(2) for j in range(2)
             for s in range(2)], axis=1)
        cb[0:KEEP, 1792:2816] = liwcat
        cb[KEEP:P, 1792:2816] = liwcat       # duplicate for base-64 operand
        f8 = ml_dtypes.float8_e4m3
        cbf8 = np.zeros((P, 768), dtype=f8)
        for si in range(2):
            for wh in range(2):
                cbf8[:, si * 256 + wh * P:si * 256 + (wh + 1) * P] = \
                    lbw[wh, si].astype(np.float32).astype(f8)
        for hh in range(2):
            cbf8[:, 512 + hh * P:512 + (hh + 1) * P] = \
                ffwd[hh].astype(np.float32).astype(f8)
        bias = np.zeros((P, 3), dtype=np.float32)
        bias[0:HID, 0] = b1n[0]
        bias[0:HID, 1] = b1n[1]
        bias[:, 2] = np.concatenate([b2n[0], b2n[1]])
        in_maps.append({
            "xbf": xs.astype(bf),
            "cb": cb,
            "bias": bias,
            "cbf8": cbf8,
        })

    return in_maps


def kernel(x, w1, b1, w2, b2):
    global _CACHED_NC
    if _CACHED_NC is None:
        _CACHED_NC = _build_nc()
    nc = _CACHED_NC
    in_maps = _prepare_in_maps(x, w1, b1, w2, b2)
    res = run_bass_kernel_spmd(nc, in_maps, list(range(NB)))
    return np.concatenate(
        [res.results[i]["out"].astype(np.float32) for i in range(NB)], axis=-1)


# revision 37
# speedup vs baseline: 1.2265x; 1.2265x over previous
"""DPOT2D layer (AFNO-style) Trainium2 kernel.

out = x + irfft2_pad(blockMLP(trunc64(rfft2(x))))   (ortho norm)

Sharding: tensor-parallel over the 8 block-diagonal channel groups — core n
gets channels [n*64, (n+1)*64) and its block's MLP weights. Blocks never mix,
so there is zero cross-core communication.

Per core, every FFT stage is a DFT matmul on the TensorEngine (bf16 operands,
fp32 PSUM accumulation), with PE-transpose corner turns between stages:

  A:  U[k1s,(w,c)]      = F_h^T @ x        (contract h, 2x128 K-chunks)
  t1: V[w,(k1s,c)]      = corner turn of U (per-channel 128x128 transposes)
  B:  Y[k2s,(k1,c)]     = DFT_w on complex U (re/im column accumulation)
  t2: Yt[(k1b,c),...]   = PAIRED turn: each transpose takes a k1 pair, so
                          all 128 out partitions are used (k1'=k1b*32+pair)
  L1: o1 = gelu(W1 Y + b1)                 (K=64 at base 0/64 per k1-bit)
  L2: O2[(o2r|o2i),(k1',k2)] = W2 o1 + b2  (K=128)
  t3: R[(k1''b,k2),...] = paired turn of O2 (k1'' = second re-pairing)
  iW: G[w,(j,k1'',c')]  = hermitian irfft_w matmuls (re/im accumulation)
  t4: Ght[(j,k1''),(w,c')] = corner turn of G (re/im -> k1-stack)
  iH: x'[h,(w,c')]      = lih_perm^T @ Ght (+ residual), DMA out

The double k1 re-pairing of t2/t3 is undone on the host by permuting the
rows of lih (k1 = g(g(k1'')), g(v) = 2*(v%32) + v//32). The residual add is
split: half the output tiles via DVE tensor_tensor, half via PE
identity-matmul accumulate + ACT copy, to balance the three engines.
Evacuations use 2-bank (1024-col) PSUM tiles; PSUM -> SBUF copies rotate
DVE/ACT; input loads split across both HWDGE rings; stores go on the SWDGE
(gpsimd) queue; all matrix constants arrive in a single packed DMA.

I/O precision: x is loaded once as bf16 (8 MB/core) and kept resident for the
residual add; the output is stored as bf16 (8 MB/core) and upcast to f32 on
the host. The spectral correction is ~3e-5 of the output norm, so total
rel-err is dominated by bf16 rounding of x (~2e-3), well inside the 2e-2 gate.
"""

import numpy as np
import ml_dtypes

import concourse.bass as bass
import concourse.mybir as mybir
from concourse import bacc
from concourse import masks
from concourse.tile import TileContext
from concourse.bass_utils import run_bass_kernel_spmd

B = 2
H = 256
W = 256
C = 512
NB = 8
BS = 64          # channels per block (= per core)
KEEP = 64        # kept modes per spatial dim
HID = 128
P = 128

BF16 = mybir.dt.bfloat16
FP8 = mybir.dt.float8e4
F32 = mybir.dt.float32
AF = mybir.ActivationFunctionType

_CACHED_NC = None


def _host_consts():
    """DFT matrices shared by all cores (fp32 -> bf16)."""
    h = np.arange(H, dtype=np.float64)[:, None]
    k = np.arange(KEEP, dtype=np.float64)[None, :]
    th = 2.0 * np.pi * h * k / H
    F = np.concatenate([np.cos(th), -np.sin(th)], axis=1) / 16.0      # (256,128)
    Fwre, Fwim = F[:, :KEEP], F[:, KEEP:]
    lb_re = np.concatenate([Fwre, Fwim], axis=1)                      # (256,128)
    lb_im = np.concatenate([-Fwim, Fwre], axis=1)
    alpha = np.where(np.arange(KEEP) == 0, 1.0, 2.0)
    k2 = np.arange(KEEP, dtype=np.float64)[:, None]
    wv = np.arange(W, dtype=np.float64)[None, :]
    tw = 2.0 * np.pi * k2 * wv / W
    Ca = alpha[:, None] * np.cos(tw) / 16.0                           # (64,256)
    Sa = alpha[:, None] * np.sin(tw) / 16.0
    k1 = np.arange(KEEP, dtype=np.float64)[:, None]
    hv = np.arange(H, dtype=np.float64)[None, :]
    tih = 2.0 * np.pi * k1 * hv / H
    Ehc = np.cos(tih) / 16.0                                          # (64,256)
    Ehs = np.sin(tih) / 16.0
    lih_full = np.concatenate([Ehc, -Ehs], axis=0)                    # (128,256)

    bf = ml_dtypes.bfloat16
    ffwd = np.stack([F[0:128], F[128:256]]).astype(bf)                # (2,128,128)
    lbw = np.stack([
        np.stack([lb_re[0:128], lb_im[0:128]]),
        np.stack([lb_re[128:256], lb_im[128:256]]),
    ]).astype(bf)                                                     # (2,2,128,128)
    liw = np.stack([
        np.stack([np.stack([Ca[:, 0:128], -Sa[:, 0:128]]),
                  np.stack([Sa[:, 0:128], Ca[:, 0:128]])]),
        np.stack([np.stack([Ca[:, 128:256], -Sa[:, 128:256]]),
                  np.stack([Sa[:, 128:256], Ca[:, 128:256]])]),
    ]).astype(bf)                                                     # (2,2,2,64,128)
    lih = np.stack([lih_full[:, 0:128], lih_full[:, 128:256]]).astype(bf)  # (2,128,128)
    return ffwd, lbw, liw, lih


def _build_nc(loop_iters=0, probe=None):
    """loop_iters>0 wraps the whole per-batch pipeline in an on-device
    For_i repeat loop — used only by the timing harness to amortize the
    ~80ms axon dispatch overhead out of the measurement.
    probe: None | 'dma' (DMAs only) | 'compute' (token input/output DMAs)."""
    nc = bacc.Bacc()

    xbf = nc.declare_dram_parameter("xbf", [B, H, W, BS], BF16, isOutput=False)
    # all bf16 matrix constants packed into one tensor -> one DMA:
    # cols 0:256 FW, 256:768 LBW, 768:1024 M2, 1024:1280 LIH (128 rows);
    # cols 1280:1792 M1, 1792:2816 LIW (rows 0:64).
    cb_d = nc.declare_dram_parameter("cb", [P, 2816], BF16, isOutput=False)
    bias_d = nc.declare_dram_parameter("bias", [P, 3], F32, isOutput=False)
    cbf8_d = nc.declare_dram_parameter("cbf8", [P, 768], FP8, isOutput=False)
    out = nc.declare_dram_parameter("out", [B, H, W, BS], BF16, isOutput=True)

    with TileContext(nc) as tc:
        consts = tc.alloc_tile_pool(name="consts", bufs=1)
        ident = consts.tile([P, P], BF16, name="ident")
        masks.make_identity(nc, ident[:])

        # consts go on the scalar ring so they overlap the first x loads
        # (which go on the sync ring).
        cb = consts.tile([P, 2816], BF16, name="cb")
        nc.scalar.dma_start(out=cb[:], in_=cb_d[:])
        bias_t = consts.tile([P, 3], F32, name="bias")
        nc.scalar.dma_start(out=bias_t[:], in_=bias_d[:])
        cbf8 = consts.tile([P, 768], FP8, name="cbf8")
        nc.scalar.dma_start(out=cbf8[:], in_=cbf8_d[:])

        FW = [cb[:, hh * P:(hh + 1) * P] for hh in range(2)]
        LBW = [[cb[:, 256 + (wh * 2 + s) * P:256 + (wh * 2 + s + 1) * P]
                for s in range(2)] for wh in range(2)]
        LBW8 = [cbf8[:, s * 256:(s + 1) * 256].rearrange("p (t m) -> p t m", t=2)
                for s in range(2)]
        FW8 = cbf8[:, 512:768].rearrange("p (t m) -> p t m", t=2)
        M2 = [cb[:, 768 + s * P:768 + (s + 1) * P] for s in range(2)]
        LIH = [cb[:, 1024 + hc * P:1024 + (hc + 1) * P] for hc in range(2)]
        # M1/LIW are stored twice (rows 0:64 and 64:128) so the stationary
        # operand can sit at either partition base for the paired corner
        # turns; index [k1bit] selects the base.
        M1 = [[[cb[bit * BS:(bit + 1) * BS,
                   1280 + (j * 2 + s) * P:1280 + (j * 2 + s + 1) * P]
                for bit in range(2)] for s in range(2)] for j in range(2)]
        LIW = [[[[cb[bit * KEEP:(bit + 1) * KEEP,
                     1792 + ((wh * 2 + j) * 2 + s) * P:
                     1792 + ((wh * 2 + j) * 2 + s + 1) * P]
                  for bit in range(2)] for s in range(2)]
                 for j in range(2)] for wh in range(2)]
        b1s_t = [bias_t[0:HID, j:j + 1] for j in range(2)]
        b2s_t = bias_t[:, 2:3]

        # copy-engine rotation (PSUM-capable engines only: DVE + ACT).
        # DVE also carries the invH residual adds, ACT the gelu/bias work.
        cp_cnt = [0]
        CP_PATTERN = globals().get("CP_PATTERN_OVERRIDE") or ("v", "a")

        def cp(dst, src):
            kind = CP_PATTERN[cp_cnt[0] % len(CP_PATTERN)]
            cp_cnt[0] += 1
            if kind == "v":
                nc.vector.tensor_copy(out=dst, in_=src)
            else:
                nc.scalar.activation(out=dst, in_=src, func=AF.Copy)

        # Tag-sharing across stage lifetimes keeps SBUF within budget:
        #   tagA/tagB: U[wh] -> G[wh]   tagC/tagD: V[wh] -> Ght[wh]
        #   tagE: Y -> R                tagF: Yt -> O2
        sb = tc.alloc_tile_pool(name="sb", bufs=1)
        xin = tc.alloc_tile_pool(name="xin", bufs=1)
        outp = tc.alloc_tile_pool(name="outp", bufs=2)
        pmm = tc.alloc_tile_pool(name="pmm", bufs=3, space="PSUM")
        ptp = tc.alloc_tile_pool(name="ptp", bufs=2, space="PSUM")

        import contextlib
        loop_ctx = tc.For_i(0, loop_iters, 1) if loop_iters else contextlib.nullcontext()
        with loop_ctx:
            _emit_body(nc, tc, locals(), probe=probe)
        ptp.release()
        pmm.release()
        outp.release()
        xin.release()
        sb.release()
        consts.release()
    nc.compile()
    return nc


_RES_MODE = (True, False)


def _emit_body(nc, tc, env, probe=None):
    xbf = env["xbf"]; out = env["out"]
    FW = env["FW"]; LBW = env["LBW"]; LBW8 = env["LBW8"]; FW8 = env["FW8"]
    M1 = env["M1"]; M2 = env["M2"]
    LIW = env["LIW"]; LIH = env["LIH"]; b1s_t = env["b1s_t"]; b2s_t = env["b2s_t"]
    ident = env["ident"]; cp = env["cp"]; cp_cnt = env["cp_cnt"]
    sb = env["sb"]; xin = env["xin"]; outp = env["outp"]
    pmm = env["pmm"]; ptp = env["ptp"]

    dma_only = probe == "dma"
    no_io = probe == "compute"

    for b in range(B):
        # -------- input loads: 8 x 1MB [128h, 64w, 64c], resident all batch.
        # wq-major order and hh split across the two HWDGE rings so stage A
        # can start after the first (wq=0) pair lands.
        xt = [[None] * 4 for _ in range(2)]
        for wq in range(4):
            for hh in range(2):
                t = xin.tile([P, 64, BS], BF16, tag=f"xin{hh}{wq}",
                             bufs=2 if wq == 0 else None,
                             name=f"xin{hh}{wq}_{b}")
                eng = nc.sync if hh == 0 else nc.scalar
                if no_io:
                    eng.dma_start(out=t[0:1, 0:1, :], in_=xbf[b, 0:1, 0:1, :])
                else:
                    eng.dma_start(
                        out=t[:],
                        in_=xbf[b, hh * P:(hh + 1) * P, wq * 64:(wq + 1) * 64, :])
                xt[hh][wq] = t
        if dma_only:
            # stores fed straight from the input tiles: same DMA traffic,
            # no compute.
            for hc in range(2):
                for wq in range(4):
                    nc.gpsimd.dma_start(
                        out=out[b, hc * P:(hc + 1) * P, wq * 64:(wq + 1) * 64, :],
                        in_=xt[hc][wq][:])
            continue

        # ---------------- stage A: U[wh] (128=k1s, (w 128, c 64)) ----------
        # 2-bank PSUM tiles: two N=512 accumulation groups -> one 1024-col
        # evacuation.
        U = [sb.tile([P, 128, BS], BF16, tag=f"tagAB{wh}", name=f"U{wh}_{b}")
             for wh in range(2)]
        for wq in range(4):          # w chunks of 64
            for mm in range(4):      # 16 w each -> 1024 cols
                ps = pmm.tile([P, 2, 8, BS], F32, tag="mm",
                              name=f"psA_{b}_{wq}_{mm}")
                for hh in range(2):  # 2 LDW per tile: reuse stationary
                    for half in range(2):
                        w0 = (mm * 2 + half) * 8
                        nc.tensor.matmul(
                            ps[:, half, :, :], FW[hh],
                            xt[hh][wq][:, w0:w0 + 8, :],
                            start=(hh == 0), stop=(hh == 1))
                wg = wq * 2 + mm // 2            # 16-w group (0..7)
                cp(U[wg // 4][:, (wg % 4) * 32 + (mm % 2) * 16:
                              (wg % 4) * 32 + (mm % 2) * 16 + 16, :], ps[:])

        # ---------------- turn1: V (128=w, (wh 2, k1s 128, c 64), fp8) -----
        # 8 transposes share one 1-bank bf16 PSUM tile -> 1024-col evac.
        # V is one fp8 tile; the wh axis is the DoubleRow k-tile dim of
        # stage B, and the copy casts bf16 -> fp8 for free.
        V = sb.tile([P, 2, P, BS], FP8, tag="tagCD0", name=f"V_{b}")
        for cg in range(8):          # groups of 8 channels; wh inner so that
            for wh in range(2):      # stage B's c-chunks unblock early
                pt = ptp.tile([P, 8, P], BF16, tag="tp", name=f"t1_{b}_{wh}_{cg}")
                for i in range(8):
                    nc.tensor.transpose(pt[:, i, :], U[wh][:, :, cg * 8 + i],
                                        ident[:])
                cp(V[:, wh, :, cg * 8:cg * 8 + 8],
                   pt[:, :, :].rearrange("a c k -> a k c"))

        # ---------------- stage B: Y (128=k2s, (k1 64, c 64)) --------------
        Y = sb.tile([P, KEEP, BS], BF16, tag="tagE", name=f"Y_{b}")
        for mm in range(4):          # 16 k1 per tile -> 1024 cols
            ps = pmm.tile([P, 2, 8, BS], F32, tag="mm", name=f"psB_{b}_{mm}")
            for s in range(2):       # 0: re rows (k1s 0:64), 1: im rows
                for half in range(2):
                    k0 = (mm * 2 + half) * 8
                    rhs = V[:, :, s * KEEP + k0:s * KEEP + k0 + 8, :]
                    nc.tensor.matmul(ps[:, half, :, :], LBW8[s], rhs,
                                     start=(s == 0), stop=(s == 1),
                                     perf_mode=mybir.MatmulPerfMode.DoubleRow)
            cp(Y[:, mm * 16:(mm + 1) * 16, :], ps[:])

        # ---------------- turn2: Yt ((k1bit,c)=128, (k1pair 32, k2s 128)) --
        # Full-width turns: each transpose takes a k1 PAIR (free dims
        # (k1 2, c 64) -> 128 out partitions). Storage k1' = k1bit*32+pair,
        # true k1 = 2*pair + k1bit; undone by the host-side lih permutation.
        Yt = sb.tile([P, 32, P], BF16, tag="tagF", name=f"Yt_{b}")
        Yf = Y[:, :, :].rearrange("a k c -> a (k c)")
        for kg in range(4):          # groups of 8 pairs
            pt = ptp.tile([P, 8, P], BF16, tag="tp", name=f"t2_{b}_{kg}")
            for i in range(8):
                q = kg * 8 + i
                nc.tensor.transpose(pt[:, i, :], Yf[:, q * P:(q + 1) * P],
                                    ident[:])
            cp(Yt[:, kg * 8:kg * 8 + 8, :], pt[:])

        # ---------------- MLP L1 (K=64) + gelu -----------------------------
        # o1 storage index k1' = k1bit*32 + pair.
        o1 = [sb.tile([HID, 2, 32, KEEP], BF16, tag=f"o1_{j}", name=f"o1_{j}_{b}")
              for j in range(2)]
        for j in range(2):
            for bit in range(2):
                for mm in range(2):  # 16 pairs per tile -> 1024 cols
                    ps = pmm.tile([HID, 2, 8, KEEP], F32, tag="mm",
                                  name=f"ps1_{b}_{j}_{bit}_{mm}")
                    for s in range(2):
                        for half in range(2):
                            p0 = (mm * 2 + half) * 8
                            nc.tensor.matmul(
                                ps[:, half, :, :], M1[j][s][bit],
                                Yt[bit * BS:(bit + 1) * BS, p0:p0 + 8,
                                   s * KEEP:(s + 1) * KEEP],
                                start=(s == 0), stop=(s == 1))
                    nc.scalar.activation(
                        out=o1[j][:, bit, mm * 16:(mm + 1) * 16, :],
                        in_=ps[:], func=AF.Gelu, bias=b1s_t[j])

        # ---------------- MLP L2 (K=128) + bias ----------------------------
        O2 = sb.tile([P, 2, 32, KEEP], BF16, tag="tagF", name=f"O2_{b}")
        for bit in range(2):
            for mm in range(2):
                ps = pmm.tile([P, 2, 8, KEEP], F32, tag="mm",
                              name=f"ps2_{b}_{bit}_{mm}")
                for j in range(2):
                    for half in range(2):
                        p0 = (mm * 2 + half) * 8
                        nc.tensor.matmul(ps[:, half, :, :], M2[j],
                                         o1[j][:, bit, p0:p0 + 8, :],
                                         start=(j == 0), stop=(j == 1))
                if cp_cnt[0] % 2 == 0:
                    nc.vector.tensor_scalar_add(
                        out=O2[:, bit, mm * 16:(mm + 1) * 16, :], in0=ps[:],
                        scalar1=b2s_t)
                else:
                    nc.scalar.activation(
                        out=O2[:, bit, mm * 16:(mm + 1) * 16, :],
                        in_=ps[:], func=AF.Identity, bias=b2s_t)
                cp_cnt[0] += 1

        # ---------------- turn3: R ((k1''bit,k2)=128, (u 32, o2s 128)) -----
        # Pairs along flat k1': storage k1'' = bit*32 + u, k1' = 2u + bit.
        R = sb.tile([P, 32, P], BF16, tag="tagE", name=f"R_{b}")
        O2f = O2[:, :, :, :].rearrange("a b p k -> a (b p k)")
        for kg in range(4):
            pt = ptp.tile([P, 8, P], BF16, tag="tp", name=f"t3_{b}_{kg}")
            for i in range(8):
                u = kg * 8 + i
                nc.tensor.transpose(pt[:, i, :], O2f[:, u * P:(u + 1) * P],
                                    ident[:])
            cp(R[:, kg * 8:kg * 8 + 8, :], pt[:])

        # ---------------- invW: G[wh] (128=w, (j 2, k1'' 64, c' 64)) -------
        G = [sb.tile([P, 2, KEEP, BS], BF16, tag=f"tagAB{wh}", name=f"G{wh}_{b}")
             for wh in range(2)]
        for wh in range(2):
            for j in range(2):       # 0: Gre, 1: Gim
                for mm in range(4):  # 16 k1'' per tile; bit = mm//2
                    bit = mm // 2
                    ps = pmm.tile([P, 2, 8, BS], F32, tag="mm",
                                  name=f"psW_{b}_{wh}_{j}_{mm}")
                    for s in range(2):
                        for half in range(2):
                            u0 = (mm % 2) * 16 + half * 8
                            nc.tensor.matmul(
                                ps[:, half, :, :], LIW[wh][j][s][bit],
                                R[bit * KEEP:(bit + 1) * KEEP, u0:u0 + 8,
                                  s * KEEP:(s + 1) * KEEP],
                                start=(s == 0), stop=(s == 1))
                    cp(G[wh][:, j, mm * 16:(mm + 1) * 16, :], ps[:])

        # ---------------- turn4: Ght (128=k1s, (w 256, c' 64)) -------------
        Ght = [sb.tile([P, P, BS], BF16, tag=f"tagCD{wh}", name=f"Ght{wh}_{b}")
               for wh in range(2)]
        for wh in range(2):
            for cg in range(8):
                pt = ptp.tile([P, 8, P], BF16, tag="tp", name=f"t4_{b}_{wh}_{cg}")
                for i in range(8):
                    # free slice (j 2, k1 64) -> out partitions [k1re | k1im]
                    nc.tensor.transpose(pt[:, i, :], G[wh][:, :, :, cg * 8 + i],
                                        ident[:])
                cp(Ght[wh][:, :, cg * 8:cg * 8 + 8], pt[:, :, :].rearrange(
                    "a b c -> a c b"))

        # ---------------- invH + residual + store --------------------------
        # Residual is added on the PE (identity-matmul accumulate), so the
        # evacuation is a plain copy that rotates across DVE/ACT. wq-major,
        # hc-inner order frees both residual tiles of a wq early, unblocking
        # the next batch's input loads.
        for wq2 in range(8):         # groups of 32 w -> 512KB stores
            wq = wq2 // 2
            for hc in range(2):
                ot = outp.tile([P, 32, BS], BF16, tag="ot",
                               name=f"ot_{b}_{hc}_{wq2}")
                for mm in range(2):  # 16 w per tile -> 1024 cols
                    gmm = (wq2 % 2) * 2 + mm       # 16-w group within wq
                    ps = pmm.tile([P, 2, 8, BS], F32, tag="mm",
                                  name=f"psH_{b}_{hc}_{wq2}_{mm}")
                    # alternate residual-add strategy: DVE tensor_tensor for
                    # half the tiles; PE identity-accumulate + ACT copy for
                    # the other half (balances PE vs DVE vs ACT).
                    on_dve = _RES_MODE[(gmm + hc) % len(_RES_MODE)]
                    for half in range(2):
                        wg = wq * 8 + gmm * 2 + half  # global 8-w group
                        nc.tensor.matmul(
                            ps[:, half, :, :], LIH[hc],
                            Ght[wg // 16][:, (wg % 16) * 8:(wg % 16) * 8 + 8, :],
                            start=True, stop=on_dve)
                    if on_dve:
                        nc.vector.tensor_tensor(
                            out=ot[:, mm * 16:(mm + 1) * 16, :], in0=ps[:],
                            in1=xt[hc][wq][:, gmm * 16:(gmm + 1) * 16, :],
                            op=mybir.AluOpType.add)
                    else:
                        for half in range(2):
                            nc.tensor.matmul(
                                ps[:, half, :, :], ident[:],
                                xt[hc][wq][:, (gmm * 2 + half) * 8:
                                           (gmm * 2 + half + 1) * 8, :],
                                start=False, stop=True)
                        nc.scalar.activation(
                            out=ot[:, mm * 16:(mm + 1) * 16, :], in_=ps[:],
                            func=AF.Copy)
                if no_io:
                    nc.gpsimd.dma_start(out=out[b, 0:1, 0:1, :],
                                        in_=ot[0:1, 0:1, :])
                else:
                    nc.gpsimd.dma_start(
                        out=out[b, hc * P:(hc + 1) * P,
                                wq2 * 32:(wq2 + 1) * 32, :],
                        in_=ot[:])


def _prepare_in_maps(x, w1, b1, w2, b2):
    bf = ml_dtypes.bfloat16
    ffwd, lbw, liw, lih = _host_consts()
    x = np.asarray(x, dtype=np.float32)

    in_maps = []
    for n in range(NB):
        xs = np.ascontiguousarray(x[..., n * BS:(n + 1) * BS])
        w1n = np.asarray(w1[:, n], dtype=np.float32)   # (2,64,128)
        w2n = np.asarray(w2[:, n], dtype=np.float32)   # (2,128,64)
        b1n = np.asarray(b1[:, n], dtype=np.float32)   # (2,128)
        b2n = np.asarray(b2[:, n], dtype=np.float32)   # (2,64)
        m1 = np.stack([
            np.stack([w1n[0], -w1n[1]]),
            np.stack([w1n[1], w1n[0]]),
        ]).astype(bf)                                   # (2,2,64,128)
        m2 = np.stack([
            np.concatenate([w2n[0], w2n[1]], axis=1),
            np.concatenate([-w2n[1], w2n[0]], axis=1),
        ]).astype(bf)                                   # (2,128,128)

        # lih rows are permuted to undo the storage order of the paired
        # corner turns: storage k1'' -> true k1 via two rounds of
        # g(v) = 2*(v%32) + v//32.
        g = lambda v: 2 * (v % 32) + v // 32
        k1true = np.array([g(g(v)) for v in range(KEEP)])
        rowperm = np.concatenate([k1true, KEEP + k1true])

        cb = np.zeros((P, 2816), dtype=bf)
        cb[:, 0:256] = np.concatenate([ffwd[0], ffwd[1]], axis=1)
        cb[:, 256:768] = np.concatenate(
            [lbw[wh, s] for wh in range(2) for s in range(2)], axis=1)
        cb[:, 768:1024] = np.concatenate([m2[0], m2[1]], axis=1)
        cb[:, 1024:1280] = np.concatenate(
            [lih[0][rowperm], lih[1][rowperm]], axis=1)
        m1cat = np.concatenate(
            [m1[j, s] for j in range(2) for s in range(2)], axis=1)
        cb[0:BS, 1280:1792] = m1cat
        cb[BS:P, 1280:1792] = m1cat          # duplicate for base-64 operand
        liwcat = np.concatenate(
            [liw[wh, j, s] for wh in range(2) for j in range(2)
             for s in range(2)], axis=1)
        cb[0:KEEP, 1792:2816] = liwcat
        cb[KEEP:P, 1792:2816] = liwcat       # duplicate for base-64 operand
        f8 = ml_dtypes.float8_e4m3
        cbf8 = np.zeros((P, 768), dtype=f8)
        for si in range(2):
            for wh in range(2):
                cbf8[:, si * 256 + wh * P:si * 256 + (wh + 1) * P] = \
                    lbw[wh, si].astype(np.float32).astype(f8)
        for hh in range(2):
            cbf8[:, 512 + hh * P:512 + (hh + 1) * P] = \
                ffwd[hh].astype(np.float32).astype(f8)
        bias = np.zeros((P, 3), dtype=np.float32)
        bias[0:HID, 0] = b1n[0]
        bias[0:HID, 1] = b1n[1]
        bias[:, 2] = np.concatenate([b2n[0], b2n[1]])
        in_maps.append({
            "xbf": xs.astype(bf),
            "cb": cb,
            "bias": bias,
            "cbf8": cbf8,
        })

    return in_maps


def kernel(x, w1, b1, w2, b2):
    global _CACHED_NC
    if _CACHED_NC is None:
        _CACHED_NC = _build_nc()
    nc = _CACHED_NC
    in_maps = _prepare_in_maps(x, w1, b1, w2, b2)
    res = run_bass_kernel_spmd(nc, in_maps, list(range(NB)))
    return np.concatenate(
        [res.results[i]["out"].astype(np.float32) for i in range(NB)], axis=-1)
